# revision 4
# baseline (speedup 1.0000x reference)
"""Trainium2 Bass kernel for nn_MultiHeadAttention_50861002719805. v2.

Full inputs in, full output out. Sharding: 8 cores = 4 batches x 2 head-groups.
Each core: 1 batch, 8 heads. Pair {2b, 2b+1} exchanges normalized per-head
outputs (bf16 O^T), each core projects all 16 heads into its 512 out columns.

v2 changes vs baseline:
- x, W_qkv host-converted to bf16; x resident in SBUF (no re-DMA per pair).
- QT/KT bf16 (halves SBUF, FWL weight loads).
- mask in bf16 so copy_predicated runs in DVE 2x mode.
- persistent psum pools sized to exactly 8 banks; QK phase of pair j+1 is
  emission-interleaved into the ACT-bound attention loop of pair j so the
  in-order PE queue has fill work while waiting on exp.
- normalize reads po psum directly (no staging copy).
- per-pair AllGather issued right after the pair's last tile; partner O
  staged into SBUF right after each collective; projection orders the
  contraction so the last pair's chunks come last.
"""
import numpy as np
import ml_dtypes

import concourse.bacc as bacc
import concourse.mybir as mybir
import concourse.tile as tile
from concourse.bass_utils import run_bass_kernel_spmd

F32 = mybir.dt.float32
F32R = mybir.dt.float32r
BF16 = mybir.dt.bfloat16
U8 = mybir.dt.uint8

B, T, D = 4, 2048, 1024
H, HS = 16, 64          # global heads, head size
HL = 8                  # heads per core
TCH, SCH = 512, 128     # t-chunk (psum free dim), s-chunk (partition tile)
NTC, NSC = T // TCH, T // SCH   # 4, 16
NDC = D // 128          # 8 contraction chunks
NP = 4                  # head pairs per core
HWID = 4 * (HS + 1)     # 260
MULT = mybir.AluOpType.mult
BYPASS = mybir.AluOpType.bypass
GROUPS = [[0, 1], [2, 3], [4, 5], [6, 7]]


def build(reps=1, collective=True):
    nc = bacc.Bacc("TRN2", target_bir_lowering=False, debug=False, num_devices=8)

    xT = nc.declare_dram_parameter("xT", [D, T], BF16, isOutput=False)
    wq = nc.declare_dram_parameter("wq", [D, HL * HS], BF16, isOutput=False)
    wk = nc.declare_dram_parameter("wk", [D, HL * HS], BF16, isOutput=False)
    wv = nc.declare_dram_parameter("wv", [D, HL * HS], BF16, isOutput=False)
    wo = nc.declare_dram_parameter("wo", [D, TCH], BF16, isOutput=False)
    mask = nc.declare_dram_parameter("mask", [4, SCH, TCH], U8, isOutput=False)
    out = nc.declare_dram_parameter("out", [T, TCH], F32, isOutput=True)

    with tile.TileContext(nc) as tc:
      for rep in range(reps):
        with (
            tc.tile_pool(name=f"const{rep}", bufs=1) as cpool,
            tc.tile_pool(name=f"wpool{rep}", bufs=1) as wpool,
            tc.tile_pool(name=f"vstp{rep}", bufs=1) as vstp,
            tc.tile_pool(name=f"small{rep}", bufs=2) as sp,
            tc.tile_pool(name=f"dram{rep}", bufs=1, space="DRAM") as dp,
        ):
            o_my = [[dp.tile([128, TCH], BF16, name=f"omy{rep}_{j}_{tcb}")
                     for tcb in range(NTC)] for j in range(NP)]
            o_all = [[dp.tile([2, 128, TCH], BF16, name=f"oall{rep}_{j}_{tcb}")
                      for tcb in range(NTC)] for j in range(NP)]

            # ---- resident x (bf16) + weights; x/wv first so V starts early ----
            x_sb = wpool.tile([128, NDC, T], BF16)
            wq_sb = wpool.tile([128, NDC, HL * HS], BF16)
            wk_sb = wpool.tile([128, NDC, HL * HS], BF16)
            wv_sb = wpool.tile([128, NDC, HL * HS], BF16)
            wo_sb = wpool.tile([128, NDC, TCH], BF16)
            for dc in range(NDC):
                nc.sync.dma_start(wv_sb[:, dc, :], wv[dc * 128:(dc + 1) * 128, :])
                nc.sync.dma_start(x_sb[:, dc, :], xT[dc * 128:(dc + 1) * 128, :])
            for dc in range(NDC):
                nc.sync.dma_start(wq_sb[:, dc, :], wq[dc * 128:(dc + 1) * 128, :])
                nc.sync.dma_start(wk_sb[:, dc, :], wk[dc * 128:(dc + 1) * 128, :])
            mask_sb = cpool.tile([SCH, 4, TCH], U8)
            for k in range(4):
                nc.sync.dma_start(mask_sb[:, k, :], mask[k, :, :])
            for dc in range(NDC):
                nc.sync.dma_start(wo_sb[:, dc, :], wo[dc * 128:(dc + 1) * 128, :])

            # ---- constants ----
            ones_col_bf = cpool.tile([128, 1], BF16)        # chunk-sum lhsT
            ones_t_bf = cpool.tile([128, TCH], BF16)        # masked-fill data
            ones_r = cpool.tile([1, TCH], F32R)             # rank-1 rhs
            nc.vector.memset(ones_col_bf[:], 1.0)
            nc.vector.memset(ones_t_bf[:], 1.0)
            nc.vector.tensor_copy(ones_r[:], ones_t_bf[0:1, :])

            # V_st[p, sc, h, 0] = 1 (Z col), cols 1:65 = v
            V_st = vstp.tile([SCH, NSC, HL, HS + 1], BF16)
            nc.vector.memset(V_st[:, :, :, 0:1], 1.0)

            # projection staging: filled per pair as collectives complete
            O_sb = vstp.tile([128, 2, NP, T], BF16)

            with (
                tc.tile_pool(name=f"spool{rep}", bufs=2, space="PSUM") as spool,
                tc.tile_pool(name=f"qpool{rep}", bufs=1, space="PSUM") as qpool,
                tc.tile_pool(name=f"opool{rep}", bufs=2, space="PSUM") as opool,
                tc.tile_pool(name=f"qkt{rep}", bufs=2) as qkt,
                tc.tile_pool(name=f"ep{rep}", bufs=2) as ep,
            ):
                # ---- V phase: dc-outer over groups of 4 s-chunks so the
                # first matmuls need only the first x chunk ----
                for grp in range(4):
                    pv = [spool.tile([SCH, 2, HL, HS], F32, tag="ps",
                                     name=f"pv{rep}_{grp}_{i}") for i in range(2)]
                    for dc in range(NDC):
                        for i in range(2):
                            for u in range(2):
                                sc = 4 * grp + 2 * i + u
                                nc.tensor.matmul(
                                    pv[i][:, u, :, :],
                                    x_sb[:, dc, sc * 128:(sc + 1) * 128],
                                    wv_sb[:, dc, :],
                                    start=(dc == 0), stop=(dc == NDC - 1),
                                    skip_group_check=True)
                    for i in range(2):
                        for u in range(2):
                            nc.vector.tensor_copy(
                                V_st[:, 4 * grp + 2 * i + u, :, 1:HS + 1],
                                pv[i][:, u, :, :])

                # ---- suffix sums incl. masked-count (col 0 of each head) ----
                vsuf_r = cpool.tile([1, 3, 2, HWID], F32R)
                for tcb in range(3):
                    for half in range(2):
                        psf = opool.tile([1, HWID], F32, tag="po",
                                         name=f"psf{rep}_{tcb}_{half}")
                        lo = 4 * (tcb + 1)
                        for c in range(lo, NSC):
                            nc.tensor.matmul(
                                psf[:], ones_col_bf[:],
                                V_st[:, c, half * 4:(half + 1) * 4, :],
                                start=(c == lo), stop=(c == NSC - 1))
                        nc.vector.tensor_copy(vsuf_r[0:1, tcb, half, :], psf[:])

                # ---- per-chunk sums for intra-block masked corrections
                vchk_b = cpool.tile([1, NSC, 2, HWID], BF16)
                for c in range(NSC):
                    if c % 4 == 0:
                        continue
                    for half in range(2):
                        pch = opool.tile([1, HWID], F32, tag="po",
                                         name=f"pch{rep}_{c}_{half}")
                        nc.tensor.matmul(
                            pch[:], ones_col_bf[:],
                            V_st[:, c, half * 4:(half + 1) * 4, :],
                            start=True, stop=True)
                        nc.vector.tensor_copy(vchk_b[0:1, c, half, :], pch[:])

                # ---- QK micro-op generator for one pair ----
                def qk_ops(j):
                    QT = qkt.tile([128, NTC, TCH], BF16, tag="qt",
                                  name=f"QT{rep}_{j}")
                    KT = qkt.tile([128, NTC, TCH], BF16, tag="kt",
                                  name=f"KT{rep}_{j}")
                    ops = []
                    state = {}
                    for tcb in range(NTC):
                        def alloc(tcb=tcb):
                            state[tcb] = (
                                qpool.tile([128, TCH], F32, tag="pq",
                                           name=f"pq{rep}_{j}_{tcb}"),
                                qpool.tile([128, TCH], F32, tag="pk",
                                           name=f"pk{rep}_{j}_{tcb}"))
                        ops.append(alloc)
                        for dc in range(NDC):
                            def mm(tcb=tcb, dc=dc):
                                pq, pk = state[tcb]
                                nc.tensor.matmul(
                                    pq[:], wq_sb[:, dc, j * 128:(j + 1) * 128],
                                    x_sb[:, dc, tcb * TCH:(tcb + 1) * TCH],
                                    start=(dc == 0), stop=(dc == NDC - 1))
                                nc.tensor.matmul(
                                    pk[:], wk_sb[:, dc, j * 128:(j + 1) * 128],
                                    x_sb[:, dc, tcb * TCH:(tcb + 1) * TCH],
                                    start=(dc == 0), stop=(dc == NDC - 1))
                            ops.append(mm)
                        def cp_out(tcb=tcb):
                            pq, pk = state[tcb]
                            nc.vector.tensor_copy(QT[:, tcb, :], pq[:])
                            nc.vector.tensor_copy(KT[:, tcb, :], pk[:])
                        ops.append(cp_out)
                    return QT, KT, ops

                def flush(ops, n=None):
                    k = len(ops) if n is None else min(n, len(ops))
                    for _ in range(k):
                        ops.pop(0)()

                QT, KT, pending = qk_ops(0)
                flush(pending)  # pair 0 QK runs standalone

                def norm_rest(j, tcb, stg):
                    # deferred tail of the normalize: off the po critical path
                    def run():
                        rp0, rbc, og = [], [], []
                        for e in range(2):
                            rp0.append(sp.tile([1, TCH], F32, tag="rp0", name=f"rp0_{j}_{tcb}_{e}"))
                            nc.vector.reciprocal(rp0[e][:], stg[e][0:1, :])
                        for e in range(2):
                            rbc.append(sp.tile([HS + 1, TCH], F32, tag="rbc", name=f"rbc_{j}_{tcb}_{e}"))
                            nc.gpsimd.partition_broadcast(
                                rbc[e][:], rp0[e][:], channels=HS + 1)
                        for e in range(2):
                            og.append(sp.tile([HS + 1, TCH], BF16, tag="og", name=f"og_{j}_{tcb}_{e}"))
                            nc.vector.tensor_tensor(
                                og[e][:], stg[e][:], rbc[e][:], MULT)
                        for e in range(2):
                            nc.sync.dma_start(
                                o_my[j][tcb][64 * e:64 * e + 64, :],
                                og[e][1:HS + 1, :])
                        # per-tcb exchange: partner data flows in while later
                        # tiles are still being computed
                        if collective:
                            nc.gpsimd.collective_compute(
                                "AllGather", BYPASS,
                                replica_groups=GROUPS,
                                ins=[o_my[j][tcb][:]],
                                outs=[o_all[j][tcb][:]],
                            )
                        for g in range(2):
                            src = (o_all[j][tcb][g, :, :] if collective
                                   else o_my[j][tcb][:])
                            nc.sync.dma_start(
                                O_sb[:, g, j, tcb * TCH:(tcb + 1) * TCH], src)
                    return run

                # projection micro-ops: one closure per 128-row t-tile
                jj_order = [g * 4 + jp for jp in range(NP) for g in range(2)]

                def proj_ops(tt):
                    def run():
                        pp = qpool.tile([128, TCH], F32,
                                        tag=("pq" if tt % 2 == 0 else "pk"),
                                        name=f"pp{rep}_{tt}")
                        for i, jj in enumerate(jj_order):
                            g, jp = jj // 4, jj % 4
                            nc.tensor.matmul(
                                pp[:],
                                O_sb[:, g, jp, tt * 128:(tt + 1) * 128],
                                wo_sb[:, jj, :],
                                start=(i == 0), stop=(i == NDC - 1))
                        ob = sp.tile([128, TCH], F32, tag="ob",
                                     name=f"ob{rep}_{tt}")
                        nc.vector.tensor_copy(ob[:], pp[:])
                        nc.sync.dma_start(out[tt * 128:(tt + 1) * 128, :],
                                          ob[:])
                    return run

                pending_norm = []
                proj_avail = []
                for j in range(NP):
                    nxt = qk_ops(j + 1) if j + 1 < NP else (None, None, [])
                    # ---- attention for heads (2j, 2j+1) ----
                    for tcb in range(NTC):
                        nv = 4 * (tcb + 1)   # valid s-chunks
                        E = [ep.tile([SCH, NSC, TCH], BF16, tag="E",
                                     name=f"E{rep}_{j}_{tcb}_{ee}")
                             for ee in range(2)]
                        po = [opool.tile([HS + 1, TCH], F32, tag="po",
                                         name=f"po{rep}_{j}_{tcb}_{ee}")
                              for ee in range(2)]
                        for cp in range(nv // 2):
                            ps = [None, None]
                            for e in range(2):
                                ps[e] = spool.tile(
                                    [SCH, 2, TCH], F32, tag="ps",
                                    name=f"ps{rep}_{j}_{tcb}_{cp}_{e}")
                            if cp == 1 and pending_norm:
                                pending_norm.pop(0)()
                                if j == NP - 1 and tcb >= 1:
                                    # pair-3 tcb-(tcb-1) exchange just issued:
                                    # its projection tiles become available
                                    proj_avail.extend(
                                        proj_ops(tt)
                                        for tt in range(4 * (tcb - 1),
                                                        4 * tcb))
                            for e in range(2):
                                for u in range(2):
                                    c = 2 * cp + u
                                    t0 = max(0, c - 4 * tcb) * 128
                                    nc.tensor.matmul(
                                        ps[e][:, u, t0:TCH],
                                        KT[64 * e:64 * e + 64, c // 4,
                                           (c % 4) * SCH:(c % 4 + 1) * SCH],
                                        QT[64 * e:64 * e + 64, tcb, t0:TCH],
                                        start=True, stop=True)
                            flush(nxt[2], 2)
                            if proj_avail:
                                proj_avail.pop(0)()
                            for e in range(2):
                                c0 = 2 * cp
                                if c0 + 1 < 4 * tcb:
                                    # both chunks fully valid: fused exp
                                    nc.scalar.activation(
                                        E[e][:, c0:c0 + 2, :], ps[e][:],
                                        mybir.ActivationFunctionType.Exp)
                                else:
                                    for u in range(2):
                                        c = c0 + u
                                        t0 = max(0, c - 4 * tcb) * 128
                                        nc.scalar.activation(
                                            E[e][:, c, t0:TCH],
                                            ps[e][:, u, t0:TCH],
                                            mybir.ActivationFunctionType.Exp)
                                for u in range(2):
                                    c = 2 * cp + u
                                    k = c - 4 * tcb
                                    if k >= 0:
                                        # boundary triangle only
                                        t0 = 128 * k
                                        nc.vector.copy_predicated(
                                            E[e][:, c, t0:t0 + 128],
                                            mask_sb[:, k, t0:t0 + 128],
                                            ones_t_bf[:, 0:128])
                            for e in range(2):
                                h = 2 * j + e
                                for u in range(2):
                                    c = 2 * cp + u
                                    t0 = max(0, c - 4 * tcb) * 128
                                    nc.tensor.matmul(
                                        po[e][:, t0:TCH],
                                        V_st[:, c, h, :],
                                        E[e][:, c, t0:TCH],
                                        start=(c == 0), stop=False,
                                        skip_group_check=True)
                            flush(nxt[2], 2)
                        # intra-block masked corrections: chunk 4tcb+k is all
                        # 1.0 for t < 128k -> rank-1 of its column sums
                        for e in range(2):
                            h = 2 * j + e
                            for k in (1, 2, 3):
                                c = 4 * tcb + k
                                nc.tensor.matmul(
                                    po[e][:, 0:128 * k],
                                    vchk_b[0:1, c, j // 2,
                                           (h % 4) * (HS + 1):
                                           (h % 4 + 1) * (HS + 1)],
                                    ones_t_bf[0:1, 0:128 * k],
                                    start=False,
                                    stop=(tcb == 3 and k == 3),
                                    skip_group_check=True)
                            if tcb < 3:
                                nc.tensor.matmul(
                                    po[e][:],
                                    vsuf_r[0:1, tcb, j // 2,
                                           (h % 4) * (HS + 1):
                                           (h % 4 + 1) * (HS + 1)],
                                    ones_r[:],
                                    start=False, stop=True,
                                    skip_group_check=True)
                        # -- normalize: fast psum->sbuf staging frees the po
                        # bank; the recip/broadcast/mult tail is deferred into
                        # the next tcb's instruction stream
                        stg = []
                        for e in range(2):
                            stg.append(sp.tile([HS + 1, TCH], F32, tag="stg", name=f"stg{rep}_{j}_{tcb}_{e}"))
                            nc.vector.tensor_copy(stg[e][:], po[e][:])
                        pending_norm.append(norm_rest(j, tcb, stg))
                    flush(nxt[2])  # leftover QK work for pair j+1
                    if j + 1 < NP:
                        QT, KT = nxt[0], nxt[1]
                while pending_norm:
                    pending_norm.pop(0)()
                while proj_avail:
                    proj_avail.pop(0)()
                for tt in range(12, T // 128):
                    proj_ops(tt)()


    nc.compile()
    return nc


def make_mask():
    # mask[k][p, f] = 1 where masked: s > t  <=>  p + 128k > f
    p = np.arange(SCH)[:, None]
    f = np.arange(TCH)[None, :]
    return np.stack([(p + 128 * k > f) for k in range(4)]).astype(np.uint8)


def make_in_maps(x, W_qkv, W_out):
    x = np.asarray(x, dtype=np.float32)
    W_qkv = np.asarray(W_qkv, dtype=np.float32)
    W_out = np.asarray(W_out, dtype=np.float32)
    mask = make_mask()
    in_maps = []
    for c in range(8):
        b, hg = c // 2, c % 2
        heads = slice(hg * HL, (hg + 1) * HL)
        # [h, d, f] -> [d, h, f] -> [d, h*f]
        wq_h = W_qkv[heads, :, 0:HS].transpose(1, 0, 2).reshape(D, HL * HS) * (1.0 / 32.0)
        wk_h = W_qkv[heads, :, HS:2 * HS].transpose(1, 0, 2).reshape(D, HL * HS)
        wv_h = W_qkv[heads, :, 2 * HS:3 * HS].transpose(1, 0, 2).reshape(D, HL * HS)
        bf = ml_dtypes.bfloat16
        in_maps.append({
            "xT": np.ascontiguousarray(x[b].T).astype(bf),
            "wq": np.ascontiguousarray(wq_h).astype(bf),
            "wk": np.ascontiguousarray(wk_h).astype(bf),
            "wv": np.ascontiguousarray(wv_h).astype(bf),
            "wo": np.ascontiguousarray(
                W_out[:, hg * TCH:(hg + 1) * TCH]).astype(bf),
            "mask": mask,
        })
    return in_maps


_NC_CACHE = {}


def get_nc():
    if "nc" not in _NC_CACHE:
        _NC_CACHE["nc"] = build()
    return _NC_CACHE["nc"]


def kernel(x, W_qkv, W_out):
    nc = get_nc()
    in_maps = make_in_maps(x, W_qkv, W_out)
    res = run_bass_kernel_spmd(nc, in_maps, list(range(8)))
    out = np.empty((B, T, D), dtype=np.float32)
    for b in range(B):
        out[b, :, 0:TCH] = res.results[2 * b]["out"]
        out[b, :, TCH:D] = res.results[2 * b + 1]["out"]
    return out


# revision 5
# speedup vs baseline: 1.0918x; 1.0918x over previous
"""Trainium2 Bass kernel for nn_MultiHeadAttention_50861002719805. v2.

Full inputs in, full output out. Sharding: 8 cores = 4 batches x 2 head-groups.
Each core: 1 batch, 8 heads. Pair {2b, 2b+1} exchanges normalized per-head
outputs (bf16 O^T), each core projects all 16 heads into its 512 out columns.

v2 changes vs baseline:
- x, W_qkv host-converted to bf16; x resident in SBUF (no re-DMA per pair).
- QT/KT bf16 (halves SBUF, FWL weight loads).
- mask in bf16 so copy_predicated runs in DVE 2x mode.
- persistent psum pools sized to exactly 8 banks; QK phase of pair j+1 is
  emission-interleaved into the ACT-bound attention loop of pair j so the
  in-order PE queue has fill work while waiting on exp.
- normalize reads po psum directly (no staging copy).
- per-pair AllGather issued right after the pair's last tile; partner O
  staged into SBUF right after each collective; projection orders the
  contraction so the last pair's chunks come last.
"""
import numpy as np
import ml_dtypes

import concourse.bacc as bacc
import concourse.mybir as mybir
import concourse.tile as tile
from concourse.bass_utils import run_bass_kernel_spmd

F32 = mybir.dt.float32
F32R = mybir.dt.float32r
BF16 = mybir.dt.bfloat16
U8 = mybir.dt.uint8

B, T, D = 4, 2048, 1024
H, HS = 16, 64          # global heads, head size
HL = 8                  # heads per core
TCH, SCH = 512, 128     # t-chunk (psum free dim), s-chunk (partition tile)
NTC, NSC = T // TCH, T // SCH   # 4, 16
NDC = D // 128          # 8 contraction chunks
NP = 4                  # head pairs per core
HWID = 4 * (HS + 1)     # 260
MULT = mybir.AluOpType.mult
BYPASS = mybir.AluOpType.bypass
GROUPS = [[0, 1], [2, 3], [4, 5], [6, 7]]


def build(reps=1, collective=True):
    nc = bacc.Bacc("TRN2", target_bir_lowering=False, debug=False, num_devices=8)

    xT = nc.declare_dram_parameter("xT", [D, T], BF16, isOutput=False)
    wq = nc.declare_dram_parameter("wq", [D, HL * HS], BF16, isOutput=False)
    wk = nc.declare_dram_parameter("wk", [D, HL * HS], BF16, isOutput=False)
    wv = nc.declare_dram_parameter("wv", [D, HL * HS], BF16, isOutput=False)
    wo = nc.declare_dram_parameter("wo", [D, TCH], BF16, isOutput=False)
    mask = nc.declare_dram_parameter("mask", [4, SCH, TCH], U8, isOutput=False)
    out = nc.declare_dram_parameter("out", [T, TCH], F32, isOutput=True)

    with tile.TileContext(nc) as tc:
      for rep in range(reps):
        with (
            tc.tile_pool(name=f"const{rep}", bufs=1) as cpool,
            tc.tile_pool(name=f"wpool{rep}", bufs=1) as wpool,
            tc.tile_pool(name=f"vstp{rep}", bufs=1) as vstp,
            tc.tile_pool(name=f"small{rep}", bufs=2) as sp,
            tc.tile_pool(name=f"dram{rep}", bufs=1, space="DRAM") as dp,
        ):
            o_my = [dp.tile([128, T], BF16, name=f"omy{rep}_{j}") for j in range(NP)]
            o_all = [dp.tile([2, 128, T], BF16, name=f"oall{rep}_{j}")
                     for j in range(NP)]

            # ---- resident x (bf16) + weights; x/wv first so V starts early ----
            x_sb = wpool.tile([128, NDC, T], BF16)
            wq_sb = wpool.tile([128, NDC, HL * HS], BF16)
            wk_sb = wpool.tile([128, NDC, HL * HS], BF16)
            wv_sb = wpool.tile([128, NDC, HL * HS], BF16)
            wo_sb = wpool.tile([128, NDC, TCH], BF16)
            for dc in range(NDC):
                nc.sync.dma_start(wv_sb[:, dc, :], wv[dc * 128:(dc + 1) * 128, :])
            for q in range(4):
                for dc in range(NDC):
                    nc.sync.dma_start(
                        x_sb[:, dc, q * TCH:(q + 1) * TCH],
                        xT[dc * 128:(dc + 1) * 128, q * TCH:(q + 1) * TCH])
            for dc in range(NDC):
                nc.sync.dma_start(wq_sb[:, dc, :], wq[dc * 128:(dc + 1) * 128, :])
                nc.sync.dma_start(wk_sb[:, dc, :], wk[dc * 128:(dc + 1) * 128, :])
            mask_sb = cpool.tile([SCH, 4, TCH], U8)
            for k in range(4):
                nc.sync.dma_start(mask_sb[:, k, :], mask[k, :, :])
            for dc in range(NDC):
                nc.sync.dma_start(wo_sb[:, dc, :], wo[dc * 128:(dc + 1) * 128, :])

            # ---- constants ----
            ones_col_bf = cpool.tile([128, 1], BF16)        # chunk-sum lhsT
            ones_t_bf = cpool.tile([128, TCH], BF16)        # masked-fill data
            ones_r = cpool.tile([1, TCH], F32R)             # rank-1 rhs
            nc.vector.memset(ones_col_bf[:], 1.0)
            nc.vector.memset(ones_t_bf[:], 1.0)
            nc.vector.tensor_copy(ones_r[:], ones_t_bf[0:1, :])

            # V_st[p, sc, h, 0] = 1 (Z col), cols 1:65 = v
            V_st = vstp.tile([SCH, NSC, HL, HS + 1], BF16)
            nc.vector.memset(V_st[:, :, :, 0:1], 1.0)

            # projection staging: filled per pair as collectives complete
            O_sb = vstp.tile([128, 2, NP, T], BF16)

            with (
                tc.tile_pool(name=f"spool{rep}", bufs=2, space="PSUM") as spool,
                tc.tile_pool(name=f"qpool{rep}", bufs=1, space="PSUM") as qpool,
                tc.tile_pool(name=f"opool{rep}", bufs=2, space="PSUM") as opool,
                tc.tile_pool(name=f"qkt{rep}", bufs=2) as qkt,
                tc.tile_pool(name=f"ep{rep}", bufs=2) as ep,
            ):
                # ---- QK micro-op generator for one pair ----
                def qk_ops(j):
                    QT = qkt.tile([128, NTC, TCH], BF16, tag="qt",
                                  name=f"QT{rep}_{j}")
                    KT = qkt.tile([128, NTC, TCH], BF16, tag="kt",
                                  name=f"KT{rep}_{j}")
                    ops = []
                    state = {}
                    for tcb in range(NTC):
                        def alloc(tcb=tcb):
                            state[tcb] = (
                                qpool.tile([128, TCH], F32, tag="pq",
                                           name=f"pq{rep}_{j}_{tcb}"),
                                qpool.tile([128, TCH], F32, tag="pk",
                                           name=f"pk{rep}_{j}_{tcb}"))
                        ops.append(alloc)
                        for dc in range(NDC):
                            def mm(tcb=tcb, dc=dc):
                                pq, pk = state[tcb]
                                nc.tensor.matmul(
                                    pq[:], wq_sb[:, dc, j * 128:(j + 1) * 128],
                                    x_sb[:, dc, tcb * TCH:(tcb + 1) * TCH],
                                    start=(dc == 0), stop=(dc == NDC - 1))
                                nc.tensor.matmul(
                                    pk[:], wk_sb[:, dc, j * 128:(j + 1) * 128],
                                    x_sb[:, dc, tcb * TCH:(tcb + 1) * TCH],
                                    start=(dc == 0), stop=(dc == NDC - 1))
                            ops.append(mm)
                        def cp_out(tcb=tcb):
                            pq, pk = state[tcb]
                            nc.vector.tensor_copy(QT[:, tcb, :], pq[:])
                            nc.vector.tensor_copy(KT[:, tcb, :], pk[:])
                        ops.append(cp_out)
                    return QT, KT, ops

                def flush(ops, n=None):
                    k = len(ops) if n is None else min(n, len(ops))
                    for _ in range(k):
                        ops.pop(0)()


                # ---- V phase: dc-outer over groups of 4 s-chunks so the
                # first matmuls need only the first x chunk ----
                for grp in range(4):
                    pv = [spool.tile([SCH, 2, HL, HS], F32, tag="ps",
                                     name=f"pv{rep}_{grp}_{i}") for i in range(2)]
                    for dc in range(NDC):
                        for i in range(2):
                            for u in range(2):
                                sc = 4 * grp + 2 * i + u
                                nc.tensor.matmul(
                                    pv[i][:, u, :, :],
                                    x_sb[:, dc, sc * 128:(sc + 1) * 128],
                                    wv_sb[:, dc, :],
                                    start=(dc == 0), stop=(dc == NDC - 1),
                                    skip_group_check=True)
                    for i in range(2):
                        for u in range(2):
                            nc.vector.tensor_copy(
                                V_st[:, 4 * grp + 2 * i + u, :, 1:HS + 1],
                                pv[i][:, u, :, :])

                # ---- suffix sums incl. masked-count (col 0 of each head) ----
                vsuf_r = cpool.tile([1, 3, 2, HWID], F32R)
                for tcb in range(3):
                    for half in range(2):
                        psf = opool.tile([1, HWID], F32, tag="po",
                                         name=f"psf{rep}_{tcb}_{half}")
                        lo = 4 * (tcb + 1)
                        for c in range(lo, NSC):
                            nc.tensor.matmul(
                                psf[:], ones_col_bf[:],
                                V_st[:, c, half * 4:(half + 1) * 4, :],
                                start=(c == lo), stop=(c == NSC - 1))
                        nc.vector.tensor_copy(vsuf_r[0:1, tcb, half, :], psf[:])

                QT, KT, pending = qk_ops(0)
                flush(pending)  # pair 0 QK runs standalone

                # ---- per-chunk sums for intra-block masked corrections
                vchk_b = cpool.tile([1, NSC, 2, HWID], BF16)
                for c in range(NSC):
                    if c % 4 == 0:
                        continue
                    for half in range(2):
                        pch = opool.tile([1, HWID], F32, tag="po",
                                         name=f"pch{rep}_{c}_{half}")
                        nc.tensor.matmul(
                            pch[:], ones_col_bf[:],
                            V_st[:, c, half * 4:(half + 1) * 4, :],
                            start=True, stop=True)
                        nc.vector.tensor_copy(vchk_b[0:1, c, half, :], pch[:])


                def norm_rest(j, tcb, stg):
                    # deferred tail of the normalize: off the po critical path
                    def run():
                        rp0, rbc, og = [], [], []
                        for e in range(2):
                            rp0.append(sp.tile([1, TCH], F32, tag="rp0", name=f"rp0_{j}_{tcb}_{e}"))
                            nc.vector.reciprocal(rp0[e][:], stg[e][0:1, :])
                        for e in range(2):
                            rbc.append(sp.tile([HS + 1, TCH], F32, tag="rbc", name=f"rbc_{j}_{tcb}_{e}"))
                            nc.gpsimd.partition_broadcast(
                                rbc[e][:], rp0[e][:], channels=HS + 1)
                        for e in range(2):
                            og.append(sp.tile([HS + 1, TCH], BF16, tag="og", name=f"og_{j}_{tcb}_{e}"))
                            nc.vector.tensor_tensor(
                                og[e][:], stg[e][:], rbc[e][:], MULT)
                        for e in range(2):
                            nc.sync.dma_start(
                                o_my[j][64 * e:64 * e + 64,
                                        tcb * TCH:(tcb + 1) * TCH],
                                og[e][1:HS + 1, :])
                        if tcb == NTC - 1:
                            # whole pair ready: one exchange per pair (HW
                            # collectives have a large fixed rendezvous cost)
                            if collective:
                                nc.gpsimd.collective_compute(
                                    "AllGather", BYPASS,
                                    replica_groups=GROUPS,
                                    ins=[o_my[j][:]],
                                    outs=[o_all[j][:]],
                                )
                            for g in range(2):
                                src = (o_all[j][g, :, :] if collective
                                       else o_my[j][:])
                                nc.sync.dma_start(O_sb[:, g, j, :], src)
                    return run

                # projection micro-ops: one closure per 128-row t-tile
                jj_order = [g * 4 + jp for jp in range(NP) for g in range(2)]

                def proj_ops(tt):
                    def run():
                        pp = qpool.tile([128, TCH], F32,
                                        tag=("pq" if tt % 2 == 0 else "pk"),
                                        name=f"pp{rep}_{tt}")
                        for i, jj in enumerate(jj_order):
                            g, jp = jj // 4, jj % 4
                            nc.tensor.matmul(
                                pp[:],
                                O_sb[:, g, jp, tt * 128:(tt + 1) * 128],
                                wo_sb[:, jj, :],
                                start=(i == 0), stop=(i == NDC - 1))
                        ob = sp.tile([128, TCH], F32, tag="ob",
                                     name=f"ob{rep}_{tt}")
                        nc.vector.tensor_copy(ob[:], pp[:])
                        nc.sync.dma_start(out[tt * 128:(tt + 1) * 128, :],
                                          ob[:])
                    return run

                pending_norm = []
                for j in range(NP):
                    nxt = qk_ops(j + 1) if j + 1 < NP else (None, None, [])
                    # ---- attention for heads (2j, 2j+1) ----
                    for tcb in range(NTC):
                        nv = 4 * (tcb + 1)   # valid s-chunks
                        E = [ep.tile([SCH, NSC, TCH], BF16, tag="E",
                                     name=f"E{rep}_{j}_{tcb}_{ee}")
                             for ee in range(2)]
                        po = [opool.tile([HS + 1, TCH], F32, tag="po",
                                         name=f"po{rep}_{j}_{tcb}_{ee}")
                              for ee in range(2)]
                        for cp in range(nv // 2):
                            ps = [None, None]
                            for e in range(2):
                                ps[e] = spool.tile(
                                    [SCH, 2, TCH], F32, tag="ps",
                                    name=f"ps{rep}_{j}_{tcb}_{cp}_{e}")
                            if cp == 1 and pending_norm:
                                pending_norm.pop(0)()
                            for e in range(2):
                                for u in range(2):
                                    c = 2 * cp + u
                                    t0 = max(0, c - 4 * tcb) * 128
                                    nc.tensor.matmul(
                                        ps[e][:, u, t0:TCH],
                                        KT[64 * e:64 * e + 64, c // 4,
                                           (c % 4) * SCH:(c % 4 + 1) * SCH],
                                        QT[64 * e:64 * e + 64, tcb, t0:TCH],
                                        start=True, stop=True)
                            flush(nxt[2], 2)
                            for e in range(2):
                                c0 = 2 * cp
                                if c0 + 1 < 4 * tcb:
                                    # both chunks fully valid: fused exp
                                    nc.scalar.activation(
                                        E[e][:, c0:c0 + 2, :], ps[e][:],
                                        mybir.ActivationFunctionType.Exp)
                                else:
                                    for u in range(2):
                                        c = c0 + u
                                        t0 = max(0, c - 4 * tcb) * 128
                                        nc.scalar.activation(
                                            E[e][:, c, t0:TCH],
                                            ps[e][:, u, t0:TCH],
                                            mybir.ActivationFunctionType.Exp)
                                for u in range(2):
                                    c = 2 * cp + u
                                    k = c - 4 * tcb
                                    if k >= 0:
                                        # boundary triangle only
                                        t0 = 128 * k
                                        nc.vector.copy_predicated(
                                            E[e][:, c, t0:t0 + 128],
                                            mask_sb[:, k, t0:t0 + 128],
                                            ones_t_bf[:, 0:128])
                            for e in range(2):
                                h = 2 * j + e
                                for u in range(2):
                                    c = 2 * cp + u
                                    t0 = max(0, c - 4 * tcb) * 128
                                    nc.tensor.matmul(
                                        po[e][:, t0:TCH],
                                        V_st[:, c, h, :],
                                        E[e][:, c, t0:TCH],
                                        start=(c == 0), stop=False,
                                        skip_group_check=True)
                            flush(nxt[2], 2)
                        # intra-block masked corrections: chunk 4tcb+k is all
                        # 1.0 for t < 128k -> rank-1 of its column sums
                        for e in range(2):
                            h = 2 * j + e
                            for k in (1, 2, 3):
                                c = 4 * tcb + k
                                nc.tensor.matmul(
                                    po[e][:, 0:128 * k],
                                    vchk_b[0:1, c, j // 2,
                                           (h % 4) * (HS + 1):
                                           (h % 4 + 1) * (HS + 1)],
                                    ones_t_bf[0:1, 0:128 * k],
                                    start=False,
                                    stop=(tcb == 3 and k == 3),
                                    skip_group_check=True)
                            if tcb < 3:
                                nc.tensor.matmul(
                                    po[e][:],
                                    vsuf_r[0:1, tcb, j // 2,
                                           (h % 4) * (HS + 1):
                                           (h % 4 + 1) * (HS + 1)],
                                    ones_r[:],
                                    start=False, stop=True,
                                    skip_group_check=True)
                        # -- normalize: fast psum->sbuf staging frees the po
                        # bank; the recip/broadcast/mult tail is deferred into
                        # the next tcb's instruction stream
                        stg = []
                        for e in range(2):
                            stg.append(sp.tile([HS + 1, TCH], F32, tag="stg", name=f"stg{rep}_{j}_{tcb}_{e}"))
                            nc.vector.tensor_copy(stg[e][:], po[e][:])
                        pending_norm.append(norm_rest(j, tcb, stg))
                    flush(nxt[2])  # leftover QK work for pair j+1
                    if j + 1 < NP:
                        QT, KT = nxt[0], nxt[1]
                while pending_norm:
                    pending_norm.pop(0)()
                for tt in range(T // 128):
                    proj_ops(tt)()


    nc.compile()
    return nc


def make_mask():
    # mask[k][p, f] = 1 where masked: s > t  <=>  p + 128k > f
    p = np.arange(SCH)[:, None]
    f = np.arange(TCH)[None, :]
    return np.stack([(p + 128 * k > f) for k in range(4)]).astype(np.uint8)


def make_in_maps(x, W_qkv, W_out):
    x = np.asarray(x, dtype=np.float32)
    W_qkv = np.asarray(W_qkv, dtype=np.float32)
    W_out = np.asarray(W_out, dtype=np.float32)
    mask = make_mask()
    in_maps = []
    for c in range(8):
        b, hg = c // 2, c % 2
        heads = slice(hg * HL, (hg + 1) * HL)
        # [h, d, f] -> [d, h, f] -> [d, h*f]
        wq_h = W_qkv[heads, :, 0:HS].transpose(1, 0, 2).reshape(D, HL * HS) * (1.0 / 32.0)
        wk_h = W_qkv[heads, :, HS:2 * HS].transpose(1, 0, 2).reshape(D, HL * HS)
        wv_h = W_qkv[heads, :, 2 * HS:3 * HS].transpose(1, 0, 2).reshape(D, HL * HS)
        bf = ml_dtypes.bfloat16
        in_maps.append({
            "xT": np.ascontiguousarray(x[b].T).astype(bf),
            "wq": np.ascontiguousarray(wq_h).astype(bf),
            "wk": np.ascontiguousarray(wk_h).astype(bf),
            "wv": np.ascontiguousarray(wv_h).astype(bf),
            "wo": np.ascontiguousarray(
                W_out[:, hg * TCH:(hg + 1) * TCH]).astype(bf),
            "mask": mask,
        })
    return in_maps


_NC_CACHE = {}


def get_nc():
    if "nc" not in _NC_CACHE:
        _NC_CACHE["nc"] = build()
    return _NC_CACHE["nc"]


def kernel(x, W_qkv, W_out):
    nc = get_nc()
    in_maps = make_in_maps(x, W_qkv, W_out)
    res = run_bass_kernel_spmd(nc, in_maps, list(range(8)))
    out = np.empty((B, T, D), dtype=np.float32)
    for b in range(B):
        out[b, :, 0:TCH] = res.results[2 * b]["out"]
        out[b, :, TCH:D] = res.results[2 * b + 1]["out"]
    return out


# revision 6
# speedup vs baseline: 1.1795x; 1.0803x over previous
"""Trainium2 Bass kernel for nn_MultiHeadAttention_50861002719805. v2.

Full inputs in, full output out. Sharding: 8 cores = 4 batches x 2 head-groups.
Each core: 1 batch, 8 heads. Pair {2b, 2b+1} exchanges normalized per-head
outputs (bf16 O^T), each core projects all 16 heads into its 512 out columns.

v2 changes vs baseline:
- x, W_qkv host-converted to bf16; x resident in SBUF (no re-DMA per pair).
- QT/KT bf16 (halves SBUF, FWL weight loads).
- mask in bf16 so copy_predicated runs in DVE 2x mode.
- persistent psum pools sized to exactly 8 banks; QK phase of pair j+1 is
  emission-interleaved into the ACT-bound attention loop of pair j so the
  in-order PE queue has fill work while waiting on exp.
- normalize reads po psum directly (no staging copy).
- per-pair AllGather issued right after the pair's last tile; partner O
  staged into SBUF right after each collective; projection orders the
  contraction so the last pair's chunks come last.
"""
import numpy as np
import ml_dtypes

import concourse.bacc as bacc
import concourse.mybir as mybir
import concourse.tile as tile
from concourse.bass_utils import run_bass_kernel_spmd

F32 = mybir.dt.float32
F32R = mybir.dt.float32r
BF16 = mybir.dt.bfloat16
U8 = mybir.dt.uint8

B, T, D = 4, 2048, 1024
H, HS = 16, 64          # global heads, head size
HL = 8                  # heads per core
TCH, SCH = 512, 128     # t-chunk (psum free dim), s-chunk (partition tile)
NTC, NSC = T // TCH, T // SCH   # 4, 16
NDC = D // 128          # 8 contraction chunks
NP = 4                  # head pairs per core
HWID = 4 * (HS + 1)     # 260
MULT = mybir.AluOpType.mult
BYPASS = mybir.AluOpType.bypass
GROUPS = [[0, 1], [2, 3], [4, 5], [6, 7]]


def build(reps=1, collective=True):
    nc = bacc.Bacc("TRN2", target_bir_lowering=False, debug=False, num_devices=8)

    xT = nc.declare_dram_parameter("xT", [D, T], BF16, isOutput=False)
    wq = nc.declare_dram_parameter("wq", [D, HL * HS], BF16, isOutput=False)
    wk = nc.declare_dram_parameter("wk", [D, HL * HS], BF16, isOutput=False)
    wv = nc.declare_dram_parameter("wv", [D, HL * HS], BF16, isOutput=False)
    wo = nc.declare_dram_parameter("wo", [D, TCH], BF16, isOutput=False)
    mask = nc.declare_dram_parameter("mask", [4, SCH, TCH], U8, isOutput=False)
    out = nc.declare_dram_parameter("out", [T, TCH], F32, isOutput=True)

    with tile.TileContext(nc) as tc:
      for rep in range(reps):
        with (
            tc.tile_pool(name=f"const{rep}", bufs=1) as cpool,
            tc.tile_pool(name=f"wpool{rep}", bufs=1) as wpool,
            tc.tile_pool(name=f"vstp{rep}", bufs=1) as vstp,
            tc.tile_pool(name=f"small{rep}", bufs=2) as sp,
            tc.tile_pool(name=f"dram{rep}", bufs=1, space="DRAM") as dp,
        ):
            o_my = [dp.tile([128, T], BF16, name=f"omy{rep}_{j}") for j in range(NP)]
            o_all = [dp.tile([2, 128, T], BF16, name=f"oall{rep}_{j}")
                     for j in range(NP)]

            # ---- resident x (bf16) + weights; x/wv first so V starts early ----
            x_sb = wpool.tile([128, NDC, T], BF16)
            wq_sb = wpool.tile([128, NDC, HL * HS], BF16)
            wk_sb = wpool.tile([128, NDC, HL * HS], BF16)
            wv_sb = wpool.tile([128, NDC, HL * HS], BF16)
            wo_sb = wpool.tile([128, NDC, TCH], BF16)
            for dc in range(NDC):
                nc.sync.dma_start(wv_sb[:, dc, :], wv[dc * 128:(dc + 1) * 128, :])
                nc.sync.dma_start(x_sb[:, dc, :], xT[dc * 128:(dc + 1) * 128, :])
            for dc in range(NDC):
                nc.sync.dma_start(wq_sb[:, dc, :], wq[dc * 128:(dc + 1) * 128, :])
                nc.sync.dma_start(wk_sb[:, dc, :], wk[dc * 128:(dc + 1) * 128, :])
            mask_sb = cpool.tile([SCH, 4, TCH], U8)
            for k in range(4):
                nc.sync.dma_start(mask_sb[:, k, :], mask[k, :, :])
            for dc in range(NDC):
                nc.sync.dma_start(wo_sb[:, dc, :], wo[dc * 128:(dc + 1) * 128, :])

            # ---- constants ----
            ones_col_bf = cpool.tile([128, 1], BF16)        # chunk-sum lhsT
            ones_t_bf = cpool.tile([128, TCH], BF16)        # masked-fill data
            ones_f = cpool.tile([1, TCH], F32)
            ones_r = cpool.tile([1, TCH], F32R)             # rank-1 rhs
            nc.vector.memset(ones_col_bf[:], 1.0)
            nc.vector.memset(ones_t_bf[:], 1.0)
            nc.vector.memset(ones_f[:], 1.0)
            nc.vector.tensor_copy(ones_r[:], ones_f[:])

            # V_st[p, sc, h, 0] = 1 (Z col), cols 1:65 = v
            V_st = vstp.tile([SCH, NSC, HL, HS + 1], BF16)
            nc.vector.memset(V_st[:, :, :, 0:1], 1.0)

            # projection staging: filled per pair as collectives complete
            O_sb = vstp.tile([128, 2, NP, T], BF16)

            with (
                tc.tile_pool(name=f"spool{rep}", bufs=2, space="PSUM") as spool,
                tc.tile_pool(name=f"qpool{rep}", bufs=1, space="PSUM") as qpool,
                tc.tile_pool(name=f"opool{rep}", bufs=2, space="PSUM") as opool,
                tc.tile_pool(name=f"qkt{rep}", bufs=2) as qkt,
                tc.tile_pool(name=f"ep{rep}", bufs=3) as ep,
            ):
                # ---- V phase: dc-outer over groups of 4 s-chunks so the
                # first matmuls need only the first x chunk ----
                for grp in range(4):
                    pv = [spool.tile([SCH, 2, HL, HS], F32, tag="ps",
                                     name=f"pv{rep}_{grp}_{i}") for i in range(2)]
                    for dc in range(NDC):
                        for i in range(2):
                            for u in range(2):
                                sc = 4 * grp + 2 * i + u
                                nc.tensor.matmul(
                                    pv[i][:, u, :, :],
                                    x_sb[:, dc, sc * 128:(sc + 1) * 128],
                                    wv_sb[:, dc, :],
                                    start=(dc == 0), stop=(dc == NDC - 1),
                                    skip_group_check=True)
                    for i in range(2):
                        for u in range(2):
                            nc.vector.tensor_copy(
                                V_st[:, 4 * grp + 2 * i + u, :, 1:HS + 1],
                                pv[i][:, u, :, :])

                # ---- suffix sums incl. masked-count (col 0 of each head) ----
                vsuf_r = cpool.tile([1, 3, 2, HWID], F32R)
                for tcb in range(3):
                    for half in range(2):
                        psf = opool.tile([1, HWID], F32, tag="po",
                                         name=f"psf{rep}_{tcb}_{half}")
                        lo = 4 * (tcb + 1)
                        for c in range(lo, NSC):
                            nc.tensor.matmul(
                                psf[:], ones_col_bf[:],
                                V_st[:, c, half * 4:(half + 1) * 4, :],
                                start=(c == lo), stop=(c == NSC - 1))
                        nc.vector.tensor_copy(vsuf_r[0:1, tcb, half, :], psf[:])

                # ---- pair sums of chunks (4t+2, 4t+3) for the coarse
                # diagonal split: their t<256 region is all-ones
                vpr_b = cpool.tile([1, NTC, 2, HWID], BF16)
                for tcb in range(NTC):
                    for half in range(2):
                        ppr = opool.tile([1, HWID], F32, tag="po",
                                         name=f"ppr{rep}_{tcb}_{half}")
                        for ci, c in enumerate((4 * tcb + 2, 4 * tcb + 3)):
                            nc.tensor.matmul(
                                ppr[:], ones_col_bf[:],
                                V_st[:, c, half * 4:(half + 1) * 4, :],
                                start=(ci == 0), stop=(ci == 1))
                        nc.vector.tensor_copy(vpr_b[0:1, tcb, half, :], ppr[:])

                # ---- QK micro-op generator for one pair ----
                def qk_ops(j):
                    QT = qkt.tile([128, NTC, TCH], BF16, tag="qt",
                                  name=f"QT{rep}_{j}")
                    KT = qkt.tile([128, NTC, TCH], BF16, tag="kt",
                                  name=f"KT{rep}_{j}")
                    ops = []
                    state = {}
                    for tcb in range(NTC):
                        def alloc(tcb=tcb):
                            state[tcb] = (
                                qpool.tile([128, TCH], F32, tag="pq",
                                           name=f"pq{rep}_{j}_{tcb}"),
                                qpool.tile([128, TCH], F32, tag="pk",
                                           name=f"pk{rep}_{j}_{tcb}"))
                        ops.append(alloc)
                        for dc in range(NDC):
                            def mm(tcb=tcb, dc=dc):
                                pq, pk = state[tcb]
                                nc.tensor.matmul(
                                    pq[:], wq_sb[:, dc, j * 128:(j + 1) * 128],
                                    x_sb[:, dc, tcb * TCH:(tcb + 1) * TCH],
                                    start=(dc == 0), stop=(dc == NDC - 1))
                                nc.tensor.matmul(
                                    pk[:], wk_sb[:, dc, j * 128:(j + 1) * 128],
                                    x_sb[:, dc, tcb * TCH:(tcb + 1) * TCH],
                                    start=(dc == 0), stop=(dc == NDC - 1))
                            ops.append(mm)
                        def cp_out(tcb=tcb):
                            pq, pk = state[tcb]
                            nc.vector.tensor_copy(QT[:, tcb, :], pq[:])
                            nc.vector.tensor_copy(KT[:, tcb, :], pk[:])
                        ops.append(cp_out)
                    return QT, KT, ops

                def flush(ops, n=None):
                    k = len(ops) if n is None else min(n, len(ops))
                    for _ in range(k):
                        ops.pop(0)()

                QT, KT, pending = qk_ops(0)
                flush(pending)  # pair 0 QK runs standalone

                def norm_rest(j, tcb, stg):
                    # deferred tail of the normalize: off the po critical path
                    def run():
                        rp0, rbc, og = [], [], []
                        for e in range(2):
                            rp0.append(sp.tile([1, TCH], F32, tag="rp0", name=f"rp0_{j}_{tcb}_{e}"))
                            nc.vector.reciprocal(rp0[e][:], stg[e][0:1, :])
                        for e in range(2):
                            rbc.append(sp.tile([HS + 1, TCH], F32, tag="rbc", name=f"rbc_{j}_{tcb}_{e}"))
                            nc.gpsimd.partition_broadcast(
                                rbc[e][:], rp0[e][:], channels=HS + 1)
                        for e in range(2):
                            og.append(sp.tile([HS + 1, TCH], BF16, tag="og", name=f"og_{j}_{tcb}_{e}"))
                            nc.vector.tensor_tensor(
                                og[e][:], stg[e][:], rbc[e][:], MULT)
                        for e in range(2):
                            nc.sync.dma_start(
                                o_my[j][64 * e:64 * e + 64,
                                        tcb * TCH:(tcb + 1) * TCH],
                                og[e][1:HS + 1, :])
                    return run

                pending_norm = []
                for j in range(NP):
                    nxt = qk_ops(j + 1) if j + 1 < NP else (None, None, [])
                    # ---- attention for heads (2j, 2j+1) ----
                    for tcb in range(NTC):
                        nv = 4 * (tcb + 1)   # valid s-chunks
                        E = [ep.tile([SCH, NSC, TCH], BF16, tag="E",
                                     name=f"E{rep}_{j}_{tcb}_{ee}")
                             for ee in range(2)]
                        po = [opool.tile([HS + 1, TCH], F32, tag="po",
                                         name=f"po{rep}_{j}_{tcb}_{ee}")
                              for ee in range(2)]
                        for cp in range(nv // 2):
                            ps = [None, None]
                            for e in range(2):
                                ps[e] = spool.tile(
                                    [SCH, 2, TCH], F32, tag="ps",
                                    name=f"ps{rep}_{j}_{tcb}_{cp}_{e}")
                            if cp == 1 and pending_norm:
                                pending_norm.pop(0)()
                            # coarse diagonal split: the second diagonal
                            # cp-group only computes t >= 256; its t < 256
                            # region is all-ones, added as one rank-1 below
                            g = cp - 2 * tcb   # >= 0: diagonal cp-group
                            t0 = 256 if g == 1 else 0
                            for e in range(2):
                                for u in range(2):
                                    c = 2 * cp + u
                                    nc.tensor.matmul(
                                        ps[e][:, u, t0:TCH],
                                        KT[64 * e:64 * e + 64, c // 4,
                                           (c % 4) * SCH:(c % 4 + 1) * SCH],
                                        QT[64 * e:64 * e + 64, tcb, t0:TCH],
                                        start=True, stop=True)
                            flush(nxt[2], 2)
                            for e in range(2):
                                # one exp over both chunks
                                nc.scalar.activation(
                                    E[e][:, 2 * cp:2 * cp + 2, t0:TCH],
                                    ps[e][:, :, t0:TCH],
                                    mybir.ActivationFunctionType.Exp)
                                if g >= 0:
                                    k0, k1 = 2 * g, 2 * g + 1
                                    c0, c1 = 2 * cp, 2 * cp + 1
                                    nc.vector.copy_predicated(
                                        E[e][:, c0, t0:t0 + 128],
                                        mask_sb[:, k0, t0:t0 + 128],
                                        ones_t_bf[:, 0:128])
                                    nc.vector.copy_predicated(
                                        E[e][:, c1, t0:t0 + 256],
                                        mask_sb[:, k1, t0:t0 + 256],
                                        ones_t_bf[:, 0:256])
                            for e in range(2):
                                h = 2 * j + e
                                for u in range(2):
                                    c = 2 * cp + u
                                    nc.tensor.matmul(
                                        po[e][:, t0:TCH],
                                        V_st[:, c, h, :],
                                        E[e][:, c, t0:TCH],
                                        start=(c == 0), stop=False,
                                        skip_group_check=True)
                                if g == 1:
                                    # all-ones region of chunks (4t+2, 4t+3)
                                    nc.tensor.matmul(
                                        po[e][:, 0:256],
                                        vpr_b[0:1, tcb, j // 2,
                                              (h % 4) * (HS + 1):
                                              (h % 4 + 1) * (HS + 1)],
                                        ones_t_bf[0:1, 0:256],
                                        start=False, stop=(tcb == 3),
                                        skip_group_check=True)
                            flush(nxt[2], 2)
                        if tcb < 3:
                            for e in range(2):
                                h = 2 * j + e
                                nc.tensor.matmul(
                                    po[e][:],
                                    vsuf_r[0:1, tcb, j // 2,
                                           (h % 4) * (HS + 1):
                                           (h % 4 + 1) * (HS + 1)],
                                    ones_r[:],
                                    start=False, stop=True,
                                    skip_group_check=True)
                        # -- normalize: fast psum->sbuf staging frees the po
                        # bank; the recip/broadcast/mult tail is deferred into
                        # the next tcb's instruction stream
                        stg = []
                        for e in range(2):
                            stg.append(sp.tile([HS + 1, TCH], F32, tag="stg", name=f"stg{rep}_{j}_{tcb}_{e}"))
                            nc.vector.tensor_copy(stg[e][:], po[e][:])
                        pending_norm.append(norm_rest(j, tcb, stg))
                    while pending_norm:
                        pending_norm.pop(0)()
                    flush(nxt[2])  # leftover QK work for pair j+1
                    if j + 1 < NP:
                        QT, KT = nxt[0], nxt[1]

                    # -- exchange this pair's O^T with the partner core --
                    if collective:
                        nc.gpsimd.collective_compute(
                            "AllGather", BYPASS,
                            replica_groups=GROUPS,
                            ins=[o_my[j][:]],
                            outs=[o_all[j][:]],
                        )
                    for g in range(2):
                        src = o_all[j][g, :, :] if collective else o_my[j][:]
                        nc.sync.dma_start(O_sb[:, g, j, :], src)

            # ---- output projection: all 16 heads x my 512 out columns ----
            with (
                tc.tile_pool(name=f"outp{rep}", bufs=3) as outp,
                tc.tile_pool(name=f"pps{rep}", bufs=4, space="PSUM") as pps,
            ):
                # contraction order: pair-major so the last pair's chunks
                # (waiting on the last collective) come last
                jj_order = [g * 4 + j for j in range(NP) for g in range(2)]
                for tt in range(T // 128):
                    pp = pps.tile([128, TCH], F32, tag="pp", name=f"pp{rep}_{tt}")
                    for i, jj in enumerate(jj_order):
                        g, j = jj // 4, jj % 4
                        nc.tensor.matmul(
                            pp[:],
                            O_sb[:, g, j, tt * 128:(tt + 1) * 128],
                            wo_sb[:, jj, :],
                            start=(i == 0), stop=(i == NDC - 1))
                    ob = outp.tile([128, TCH], F32, tag="ob", name=f"ob{rep}_{tt}")
                    nc.scalar.copy(ob[:], pp[:])
                    nc.sync.dma_start(out[tt * 128:(tt + 1) * 128, :], ob[:])

    nc.compile()
    return nc


def make_mask():
    # mask[k][p, f] = 1 where masked: s > t  <=>  p + 128k > f
    p = np.arange(SCH)[:, None]
    f = np.arange(TCH)[None, :]
    return np.stack([(p + 128 * k > f) for k in range(4)]).astype(np.uint8)


def make_in_maps(x, W_qkv, W_out):
    x = np.asarray(x, dtype=np.float32)
    W_qkv = np.asarray(W_qkv, dtype=np.float32)
    W_out = np.asarray(W_out, dtype=np.float32)
    mask = make_mask()
    in_maps = []
    for c in range(8):
        b, hg = c // 2, c % 2
        heads = slice(hg * HL, (hg + 1) * HL)
        # [h, d, f] -> [d, h, f] -> [d, h*f]
        wq_h = W_qkv[heads, :, 0:HS].transpose(1, 0, 2).reshape(D, HL * HS) * (1.0 / 32.0)
        wk_h = W_qkv[heads, :, HS:2 * HS].transpose(1, 0, 2).reshape(D, HL * HS)
        wv_h = W_qkv[heads, :, 2 * HS:3 * HS].transpose(1, 0, 2).reshape(D, HL * HS)
        bf = ml_dtypes.bfloat16
        in_maps.append({
            "xT": np.ascontiguousarray(x[b].T).astype(bf),
            "wq": np.ascontiguousarray(wq_h).astype(bf),
            "wk": np.ascontiguousarray(wk_h).astype(bf),
            "wv": np.ascontiguousarray(wv_h).astype(bf),
            "wo": np.ascontiguousarray(
                W_out[:, hg * TCH:(hg + 1) * TCH]).astype(bf),
            "mask": mask,
        })
    return in_maps


_NC_CACHE = {}


def get_nc():
    if "nc" not in _NC_CACHE:
        _NC_CACHE["nc"] = build()
    return _NC_CACHE["nc"]


def kernel(x, W_qkv, W_out):
    nc = get_nc()
    in_maps = make_in_maps(x, W_qkv, W_out)
    res = run_bass_kernel_spmd(nc, in_maps, list(range(8)))
    out = np.empty((B, T, D), dtype=np.float32)
    for b in range(B):
        out[b, :, 0:TCH] = res.results[2 * b]["out"]
        out[b, :, TCH:D] = res.results[2 * b + 1]["out"]
    return out


# revision 7
# speedup vs baseline: 1.2119x; 1.0275x over previous
"""Trainium2 Bass kernel for nn_MultiHeadAttention_50861002719805. v2.

Full inputs in, full output out. Sharding: 8 cores = 4 batches x 2 head-groups.
Each core: 1 batch, 8 heads. Pair {2b, 2b+1} exchanges normalized per-head
outputs (bf16 O^T), each core projects all 16 heads into its 512 out columns.

v2 changes vs baseline:
- x, W_qkv host-converted to bf16; x resident in SBUF (no re-DMA per pair).
- QT/KT bf16 (halves SBUF, FWL weight loads).
- mask in bf16 so copy_predicated runs in DVE 2x mode.
- persistent psum pools sized to exactly 8 banks; QK phase of pair j+1 is
  emission-interleaved into the ACT-bound attention loop of pair j so the
  in-order PE queue has fill work while waiting on exp.
- normalize reads po psum directly (no staging copy).
- per-pair AllGather issued right after the pair's last tile; partner O
  staged into SBUF right after each collective; projection orders the
  contraction so the last pair's chunks come last.
"""
import numpy as np
import ml_dtypes

import concourse.bacc as bacc
import concourse.mybir as mybir
import concourse.tile as tile
from concourse.bass_utils import run_bass_kernel_spmd

F32 = mybir.dt.float32
F32R = mybir.dt.float32r
BF16 = mybir.dt.bfloat16
U8 = mybir.dt.uint8

B, T, D = 4, 2048, 1024
H, HS = 16, 64          # global heads, head size
HL = 8                  # heads per core
TCH, SCH = 512, 128     # t-chunk (psum free dim), s-chunk (partition tile)
NTC, NSC = T // TCH, T // SCH   # 4, 16
NDC = D // 128          # 8 contraction chunks
NP = 4                  # head pairs per core
HWID = 4 * (HS + 1)     # 260
MULT = mybir.AluOpType.mult
BYPASS = mybir.AluOpType.bypass
GROUPS = [[0, 1], [2, 3], [4, 5], [6, 7]]


def build(reps=1, collective=True):
    nc = bacc.Bacc("TRN2", target_bir_lowering=False, debug=False, num_devices=8)

    xT = nc.declare_dram_parameter("xT", [D, T], BF16, isOutput=False)
    wq = nc.declare_dram_parameter("wq", [D, HL * HS], BF16, isOutput=False)
    wk = nc.declare_dram_parameter("wk", [D, HL * HS], BF16, isOutput=False)
    wv = nc.declare_dram_parameter("wv", [D, HL * HS], BF16, isOutput=False)
    wo = nc.declare_dram_parameter("wo", [D, TCH], BF16, isOutput=False)
    mask = nc.declare_dram_parameter("mask", [4, SCH, TCH], U8, isOutput=False)
    out = nc.declare_dram_parameter("out", [T, TCH], BF16, isOutput=True)

    with tile.TileContext(nc) as tc:
        with (
            tc.tile_pool(name="const", bufs=1) as cpool,
            tc.tile_pool(name="wpool", bufs=1) as wpool,
            tc.tile_pool(name="vstp", bufs=1) as vstp,
            tc.tile_pool(name="small", bufs=2) as sp,
            tc.tile_pool(name="dram", bufs=1, space="DRAM") as dp,
            tc.tile_pool(name="spool", bufs=2, space="PSUM") as spool,
            tc.tile_pool(name="qpool", bufs=1, space="PSUM") as qpool,
            tc.tile_pool(name="opool", bufs=2, space="PSUM") as opool,
            tc.tile_pool(name="qkt", bufs=2) as qkt,
            tc.tile_pool(name="ep", bufs=3) as ep,
        ):
          for rep in range(reps):
            o_my = [dp.tile([128, T], BF16, name=f"omy{rep}_{j}") for j in range(NP)]
            o_all = [dp.tile([2, 128, T], BF16, name=f"oall{rep}_{j}")
                     for j in range(NP)]

            # ---- resident x (bf16) + weights; x/wv first so V starts early ----
            x_sb = wpool.tile([128, NDC, T], BF16)
            wq_sb = wpool.tile([128, NDC, HL * HS], BF16)
            wk_sb = wpool.tile([128, NDC, HL * HS], BF16)
            wv_sb = wpool.tile([128, NDC, HL * HS], BF16)
            wo_sb = wpool.tile([128, NDC, TCH], BF16)
            for dc in range(NDC):
                nc.sync.dma_start(wv_sb[:, dc, :], wv[dc * 128:(dc + 1) * 128, :])
                nc.sync.dma_start(x_sb[:, dc, :], xT[dc * 128:(dc + 1) * 128, :])
            for dc in range(NDC):
                nc.sync.dma_start(wq_sb[:, dc, :], wq[dc * 128:(dc + 1) * 128, :])
                nc.sync.dma_start(wk_sb[:, dc, :], wk[dc * 128:(dc + 1) * 128, :])
            mask_sb = cpool.tile([SCH, 4, TCH], U8)
            for k in range(4):
                nc.sync.dma_start(mask_sb[:, k, :], mask[k, :, :])
            for dc in range(NDC):
                nc.sync.dma_start(wo_sb[:, dc, :], wo[dc * 128:(dc + 1) * 128, :])

            # ---- constants ----
            ones_col_bf = cpool.tile([128, 1], BF16)        # chunk-sum lhsT
            ones_t_bf = cpool.tile([128, TCH], BF16)        # masked-fill data
            nc.vector.memset(ones_col_bf[:], 1.0)
            nc.vector.memset(ones_t_bf[:], 1.0)

            # V_st[p, sc, h, 0] = 1 (Z col), cols 1:65 = v
            V_st = vstp.tile([SCH, NSC, HL, HS + 1], BF16)
            nc.vector.memset(V_st[:, :, :, 0:1], 1.0)

            # projection staging: filled per pair as collectives complete
            O_sb = vstp.tile([128, 2, NP, T], BF16)

            if True:
                # ---- V phase: dc-outer over groups of 4 s-chunks so the
                # first matmuls need only the first x chunk ----
                for grp in range(4):
                    pv = [spool.tile([SCH, 2, HL, HS], F32, tag="ps",
                                     name=f"pv{rep}_{grp}_{i}") for i in range(2)]
                    for dc in range(NDC):
                        for i in range(2):
                            for u in range(2):
                                sc = 4 * grp + 2 * i + u
                                nc.tensor.matmul(
                                    pv[i][:, u, :, :],
                                    x_sb[:, dc, sc * 128:(sc + 1) * 128],
                                    wv_sb[:, dc, :],
                                    start=(dc == 0), stop=(dc == NDC - 1),
                                    skip_group_check=True)
                    for i in range(2):
                        for u in range(2):
                            nc.vector.tensor_copy(
                                V_st[:, 4 * grp + 2 * i + u, :, 1:HS + 1],
                                pv[i][:, u, :, :])

                # ---- suffix sums incl. masked-count (col 0 of each head) ----
                vsuf_r = cpool.tile([1, 3, 2, HWID], BF16)
                for tcb in range(3):
                    for half in range(2):
                        psf = opool.tile([1, HWID], F32, tag="po",
                                         name=f"psf{rep}_{tcb}_{half}")
                        lo = 4 * (tcb + 1)
                        for c in range(lo, NSC):
                            nc.tensor.matmul(
                                psf[:], ones_col_bf[:],
                                V_st[:, c, half * 4:(half + 1) * 4, :],
                                start=(c == lo), stop=(c == NSC - 1))
                        nc.vector.tensor_copy(vsuf_r[0:1, tcb, half, :], psf[:])

                # ---- pair sums of chunks (4t+2, 4t+3) for the coarse
                # diagonal split: their t<256 region is all-ones
                vpr_b = cpool.tile([1, NTC, 2, HWID], BF16)
                for tcb in range(NTC):
                    for half in range(2):
                        ppr = opool.tile([1, HWID], F32, tag="po",
                                         name=f"ppr{rep}_{tcb}_{half}")
                        for ci, c in enumerate((4 * tcb + 2, 4 * tcb + 3)):
                            nc.tensor.matmul(
                                ppr[:], ones_col_bf[:],
                                V_st[:, c, half * 4:(half + 1) * 4, :],
                                start=(ci == 0), stop=(ci == 1))
                        nc.vector.tensor_copy(vpr_b[0:1, tcb, half, :], ppr[:])

                # ---- QK micro-op generator for one pair ----
                def qk_ops(j):
                    QT = qkt.tile([128, NTC, TCH], BF16, tag="qt",
                                  name=f"QT{rep}_{j}")
                    KT = qkt.tile([128, NTC, TCH], BF16, tag="kt",
                                  name=f"KT{rep}_{j}")
                    ops = []
                    state = {}
                    for tcb in range(NTC):
                        def alloc(tcb=tcb):
                            state[tcb] = (
                                qpool.tile([128, TCH], F32, tag="pq",
                                           name=f"pq{rep}_{j}_{tcb}"),
                                qpool.tile([128, TCH], F32, tag="pk",
                                           name=f"pk{rep}_{j}_{tcb}"))
                        ops.append(alloc)
                        for dc in range(NDC):
                            def mm(tcb=tcb, dc=dc):
                                pq, pk = state[tcb]
                                nc.tensor.matmul(
                                    pq[:], wq_sb[:, dc, j * 128:(j + 1) * 128],
                                    x_sb[:, dc, tcb * TCH:(tcb + 1) * TCH],
                                    start=(dc == 0), stop=(dc == NDC - 1))
                                nc.tensor.matmul(
                                    pk[:], wk_sb[:, dc, j * 128:(j + 1) * 128],
                                    x_sb[:, dc, tcb * TCH:(tcb + 1) * TCH],
                                    start=(dc == 0), stop=(dc == NDC - 1))
                            ops.append(mm)
                        def cp_out(tcb=tcb):
                            pq, pk = state[tcb]
                            nc.vector.tensor_copy(QT[:, tcb, :], pq[:])
                            nc.vector.tensor_copy(KT[:, tcb, :], pk[:])
                        ops.append(cp_out)
                    return QT, KT, ops

                def flush(ops, n=None):
                    k = len(ops) if n is None else min(n, len(ops))
                    for _ in range(k):
                        ops.pop(0)()

                QT, KT, pending = qk_ops(0)
                flush(pending)  # pair 0 QK runs standalone

                def norm_rest(j, tcb, stg):
                    # deferred tail of the normalize: off the po critical path
                    def run():
                        rp0, rbc, og = [], [], []
                        for e in range(2):
                            rp0.append(sp.tile([1, TCH], F32, tag="rp0", name=f"rp0_{j}_{tcb}_{e}"))
                            nc.vector.reciprocal(rp0[e][:], stg[e][0:1, :])
                        for e in range(2):
                            rbc.append(sp.tile([HS + 1, TCH], F32, tag="rbc", name=f"rbc_{j}_{tcb}_{e}"))
                            nc.gpsimd.partition_broadcast(
                                rbc[e][:], rp0[e][:], channels=HS + 1)
                        for e in range(2):
                            og.append(sp.tile([HS + 1, TCH], BF16, tag="og", name=f"og_{j}_{tcb}_{e}"))
                            nc.vector.tensor_tensor(
                                og[e][:], stg[e][:], rbc[e][:], MULT)
                        for e in range(2):
                            nc.sync.dma_start(
                                o_my[j][64 * e:64 * e + 64,
                                        tcb * TCH:(tcb + 1) * TCH],
                                og[e][1:HS + 1, :])
                    return run

                pending_norm = []
                for j in range(NP):
                    nxt = qk_ops(j + 1) if j + 1 < NP else (None, None, [])
                    # ---- attention for heads (2j, 2j+1) ----
                    for tcb in range(NTC):
                        nv = 4 * (tcb + 1)   # valid s-chunks
                        E = [ep.tile([SCH, NSC, TCH], BF16, tag="E",
                                     name=f"E{rep}_{j}_{tcb}_{ee}")
                             for ee in range(2)]
                        po = [opool.tile([HS + 1, TCH], F32, tag="po",
                                         name=f"po{rep}_{j}_{tcb}_{ee}")
                              for ee in range(2)]
                        for cp in range(nv // 2):
                            ps = [None, None]
                            for e in range(2):
                                ps[e] = spool.tile(
                                    [SCH, 2, TCH], F32, tag="ps",
                                    name=f"ps{rep}_{j}_{tcb}_{cp}_{e}")
                            if cp == 1 and pending_norm:
                                pending_norm.pop(0)()
                            # coarse diagonal split: the second diagonal
                            # cp-group only computes t >= 256; its t < 256
                            # region is all-ones, added as one rank-1 below
                            g = cp - 2 * tcb   # >= 0: diagonal cp-group
                            t0 = 256 if g == 1 else 0
                            for u in range(2):
                                for e in range(2):
                                    c = 2 * cp + u
                                    nc.tensor.matmul(
                                        ps[e][:, u, t0:TCH],
                                        KT[64 * e:64 * e + 64, c // 4,
                                           (c % 4) * SCH:(c % 4 + 1) * SCH],
                                        QT[64 * e:64 * e + 64, tcb, t0:TCH],
                                        start=True, stop=True)
                            flush(nxt[2], 2)
                            for e in range(2):
                                # one exp over both chunks
                                nc.scalar.activation(
                                    E[e][:, 2 * cp:2 * cp + 2, t0:TCH],
                                    ps[e][:, :, t0:TCH],
                                    mybir.ActivationFunctionType.Exp)
                                if g >= 0:
                                    k0, k1 = 2 * g, 2 * g + 1
                                    c0, c1 = 2 * cp, 2 * cp + 1
                                    nc.vector.copy_predicated(
                                        E[e][:, c0, t0:t0 + 128],
                                        mask_sb[:, k0, t0:t0 + 128],
                                        ones_t_bf[:, 0:128])
                                    nc.vector.copy_predicated(
                                        E[e][:, c1, t0:t0 + 256],
                                        mask_sb[:, k1, t0:t0 + 256],
                                        ones_t_bf[:, 0:256])
                            for e in range(2):
                                h = 2 * j + e
                                for u in range(2):
                                    c = 2 * cp + u
                                    nc.tensor.matmul(
                                        po[e][:, t0:TCH],
                                        V_st[:, c, h, :],
                                        E[e][:, c, t0:TCH],
                                        start=(c == 0), stop=False,
                                        skip_group_check=True)
                                if g == 1:
                                    # all-ones region of chunks (4t+2, 4t+3)
                                    nc.tensor.matmul(
                                        po[e][:, 0:256],
                                        vpr_b[0:1, tcb, j // 2,
                                              (h % 4) * (HS + 1):
                                              (h % 4 + 1) * (HS + 1)],
                                        ones_t_bf[0:1, 0:256],
                                        start=False, stop=(tcb == 3),
                                        skip_group_check=True)
                            flush(nxt[2], 2)
                        if tcb < 3:
                            for e in range(2):
                                h = 2 * j + e
                                nc.tensor.matmul(
                                    po[e][:],
                                    vsuf_r[0:1, tcb, j // 2,
                                           (h % 4) * (HS + 1):
                                           (h % 4 + 1) * (HS + 1)],
                                    ones_t_bf[0:1, :],
                                    start=False, stop=True,
                                    skip_group_check=True)
                        # -- normalize: fast psum->sbuf staging frees the po
                        # bank; the recip/broadcast/mult tail is deferred into
                        # the next tcb's instruction stream
                        stg = []
                        for e in range(2):
                            stg.append(sp.tile([HS + 1, TCH], F32, tag="stg", name=f"stg{rep}_{j}_{tcb}_{e}"))
                            nc.vector.tensor_copy(stg[e][:], po[e][:])
                        pending_norm.append(norm_rest(j, tcb, stg))
                    while pending_norm:
                        pending_norm.pop(0)()
                    flush(nxt[2])  # leftover QK work for pair j+1
                    if j + 1 < NP:
                        QT, KT = nxt[0], nxt[1]

                    # -- exchange this pair's O^T with the partner core --
                    if collective:
                        nc.gpsimd.collective_compute(
                            "AllGather", BYPASS,
                            replica_groups=GROUPS,
                            ins=[o_my[j][:]],
                            outs=[o_all[j][:]],
                        )
                    for g in range(2):
                        src = o_all[j][g, :, :] if collective else o_my[j][:]
                        nc.sync.dma_start(O_sb[:, g, j, :], src)

                # ---- output projection: all 16 heads x my 512 cols ----
                # contraction order: pair-major so the last pair's chunks
                # (waiting on the last collective) come last
                jj_order = [g * 4 + j for j in range(NP) for g in range(2)]
                for tt in range(T // 128):
                    pp = qpool.tile([128, TCH], F32,
                                    tag=("pq" if tt % 2 == 0 else "pk"),
                                    name=f"pp{rep}_{tt}")
                    for i, jj in enumerate(jj_order):
                        g, j = jj // 4, jj % 4
                        nc.tensor.matmul(
                            pp[:],
                            O_sb[:, g, j, tt * 128:(tt + 1) * 128],
                            wo_sb[:, jj, :],
                            start=(i == 0), stop=(i == NDC - 1))
                    ob = sp.tile([128, TCH], BF16, tag="ob", name=f"ob{rep}_{tt}")
                    nc.scalar.copy(ob[:], pp[:])
                    nc.sync.dma_start(out[tt * 128:(tt + 1) * 128, :], ob[:])

    nc.compile()
    return nc


def make_mask():
    # mask[k][p, f] = 1 where masked: s > t  <=>  p + 128k > f
    p = np.arange(SCH)[:, None]
    f = np.arange(TCH)[None, :]
    return np.stack([(p + 128 * k > f) for k in range(4)]).astype(np.uint8)


def make_in_maps(x, W_qkv, W_out):
    x = np.asarray(x, dtype=np.float32)
    W_qkv = np.asarray(W_qkv, dtype=np.float32)
    W_out = np.asarray(W_out, dtype=np.float32)
    mask = make_mask()
    in_maps = []
    for c in range(8):
        b, hg = c // 2, c % 2
        heads = slice(hg * HL, (hg + 1) * HL)
        # [h, d, f] -> [d, h, f] -> [d, h*f]
        wq_h = W_qkv[heads, :, 0:HS].transpose(1, 0, 2).reshape(D, HL * HS) * (1.0 / 32.0)
        wk_h = W_qkv[heads, :, HS:2 * HS].transpose(1, 0, 2).reshape(D, HL * HS)
        wv_h = W_qkv[heads, :, 2 * HS:3 * HS].transpose(1, 0, 2).reshape(D, HL * HS)
        bf = ml_dtypes.bfloat16
        in_maps.append({
            "xT": np.ascontiguousarray(x[b].T).astype(bf),
            "wq": np.ascontiguousarray(wq_h).astype(bf),
            "wk": np.ascontiguousarray(wk_h).astype(bf),
            "wv": np.ascontiguousarray(wv_h).astype(bf),
            "wo": np.ascontiguousarray(
                W_out[:, hg * TCH:(hg + 1) * TCH]).astype(bf),
            "mask": mask,
        })
    return in_maps


_NC_CACHE = {}


def get_nc():
    if "nc" not in _NC_CACHE:
        _NC_CACHE["nc"] = build()
    return _NC_CACHE["nc"]


def kernel(x, W_qkv, W_out):
    nc = get_nc()
    in_maps = make_in_maps(x, W_qkv, W_out)
    res = run_bass_kernel_spmd(nc, in_maps, list(range(8)))
    out = np.empty((B, T, D), dtype=np.float32)
    for b in range(B):
        out[b, :, 0:TCH] = np.asarray(res.results[2 * b]["out"],
                                      dtype=np.float32)
        out[b, :, TCH:D] = np.asarray(res.results[2 * b + 1]["out"],
                                      dtype=np.float32)
    return out
